# revision 15
# baseline (speedup 1.0000x reference)
"""Distributed Dot-GAT kernel for 8 Trainium2 NeuronCores.

Sharding:
  - Per-agent weights (W_embed/W_q/W_k/W_fwd, b_fwd) sharded 16 agents/core.
  - Embed, LN, q/k/v projections, fwd MLP run agent-sharded (b-major rows
    (a_loc, b), free = d).
  - Attention is sharded by batch (4 of 32 per core): q/k (transposed to
    d-major) and v are exchanged with a single AllToAll per step; attention
    output (after LN) returns to agent sharding with a second AllToAll.
  - Projection matmuls run as float32r (1 cyc/row at N=512); attention
    matmuls stay fp32 (no speed difference at N=128).

All cores run the same program (SPMD); no core-id-dependent addressing.
"""
import numpy as np

import concourse.bass as bass
import concourse.bacc as bacc
import concourse.tile as tile
import concourse.mybir as mybir
from concourse import bass_utils
from concourse.masks import make_identity

B, A, D = 32, 128, 512
H, DH = 8, 64
NC = 8
AL = A // NC          # agents per core
BL = B // NC          # batches per core (attention phase)
STEPS = 3
KT = D // 128         # contraction tiles
RT = (AL * B) // 128  # row tiles of the (a_loc, b) x d state
EPS = 1e-5
F32 = mybir.dt.float32
F32R = mybir.dt.float32r
AF = mybir.ActivationFunctionType
ALU = mybir.AluOpType


DEBUG = False


def build(reps=1):
    nc = bacc.Bacc("TRN2", target_bir_lowering=False, debug=False,
                   num_devices=NC)
    taps = {}
    def tap(name, shape):
        if DEBUG and name not in taps:
            taps[name] = nc.dram_tensor(name, shape, F32,
                                        kind="ExternalOutput").ap()
        return taps.get(name)
    def tap_dma(name, src_ap, dst_slice=None):
        if not DEBUG:
            return
        t = taps[name]
        nc.sync.dma_start(out=t if dst_slice is None else dst_slice(t),
                          in_=src_ap)

    x_in = nc.dram_tensor("x", [B, AL, D], F32, kind="ExternalInput").ap()
    w_emb = nc.dram_tensor("w_embed", [AL, D, D], F32, kind="ExternalInput").ap()
    w_q = nc.dram_tensor("w_q", [AL, D, D], F32, kind="ExternalInput").ap()
    w_k = nc.dram_tensor("w_k", [AL, D, D], F32, kind="ExternalInput").ap()
    w_f = nc.dram_tensor("w_fwd", [AL, D, D], F32, kind="ExternalInput").ap()
    w_v = nc.dram_tensor("w_v", [D, D], F32, kind="ExternalInput").ap()
    bf_in = nc.dram_tensor("b_fwd", [AL, D], F32, kind="ExternalInput").ap()
    g0_in = nc.dram_tensor("ln0_g", [D], F32, kind="ExternalInput").ap()
    b0_in = nc.dram_tensor("ln0_b", [D], F32, kind="ExternalInput").ap()
    g1_in = nc.dram_tensor("ln1_g", [D], F32, kind="ExternalInput").ap()
    b1_in = nc.dram_tensor("ln1_b", [D], F32, kind="ExternalInput").ap()
    ct_in = nc.dram_tensor("connect_t", [A, A], F32, kind="ExternalInput").ap()
    h_out = nc.dram_tensor("h_out", [AL * B, D], F32, kind="ExternalOutput").ap()
    if True:
        tap("t_h0", [AL * B, D])
        tap("t_xn0", [AL * B, D])
        tap("t_q0", [AL * B, D])
        tap("t_k0", [AL * B, D])
        tap("t_v0", [AL * B, D])
        tap("t_qTb0", [128, KT * A])
        tap("t_kTb0", [128, KT * A])
        tap("t_vb0", [128, H * (DH + 1)])
        tap("t_eT0", [A, A])
        tap("t_ob0", [128, D])
        tap("t_xnb0", [128, D])
        tap("t_xf0", [AL * B, D])
        tap("t_st0", [AL * B, D])

    with tile.TileContext(nc) as tc:
        with (
            tc.tile_pool(name="const", bufs=1) as const,
            tc.tile_pool(name="w", bufs=3) as wpool,
            tc.tile_pool(name="wf", bufs=2) as wfpool,
            tc.tile_pool(name="big", bufs=1) as big,
            tc.tile_pool(name="xnp", bufs=2) as xnp,
            tc.tile_pool(name="att", bufs=2) as att,
            tc.tile_pool(name="sm", bufs=4) as sm,
            tc.tile_pool(name="ps", bufs=7, space="PSUM") as ps,
            tc.tile_pool(name="dram", bufs=2, space="DRAM") as dpool,
        ):
            # ---- constants ----
            ident = const.tile([128, 128], F32)
            make_identity(nc, ident)
            ct_sb = const.tile([A, A], F32)
            nc.sync.dma_start(out=ct_sb, in_=ct_in)
            bcast = []
            for t_in in (g0_in, b0_in, g1_in, b1_in):
                t_sb = const.tile([128, D], F32, tag=f"bc_{t_in.tensor.name}")
                nc.sync.dma_start(out=t_sb, in_=t_in[None].to_broadcast([128, D]))
                bcast.append(t_sb)
            g0bc, b0bc, g1bc, b1bc = bcast
            wv_t = const.tile([128, KT, D], F32R)
            nc.sync.dma_start(
                out=wv_t,
                in_=w_v.rearrange("(kt p) n -> p kt n", p=128).bitcast(F32R))
            bfs = const.tile([1, AL, D], F32R)
            nc.sync.dma_start(out=bfs, in_=bf_in[None].bitcast(F32R))
            eps_t = const.tile([128, 1], F32)
            nc.vector.memset(eps_t, EPS)
            ones_f = const.tile([1, 32], F32)
            nc.vector.memset(ones_f, 1.0)
            ones_r = const.tile([1, 32], F32R)
            nc.vector.tensor_copy(ones_r, ones_f)

            def ln_stats(src_ap):
                """Returns (mean [p,1], rstd [p,1]) tiles for ln over free dim."""
                p = src_ap.shape[0]
                st = sm.tile([128, 6], F32, tag="st")
                mv = sm.tile([128, 2], F32, tag="mv")
                sq = sm.tile([128, 1], F32, tag="sq")
                rstd = sm.tile([128, 1], F32, tag="rstd")
                st, mv, sq, rstd = st[0:p], mv[0:p], sq[0:p], rstd[0:p]
                nc.vector.bn_stats(st, src_ap)
                nc.vector.bn_aggr(mv, st)
                nc.scalar.activation(sq, mv[:, 1:2], AF.Sqrt, bias=eps_t[0:p])
                nc.vector.reciprocal(rstd, sq)
                return mv[:, 0:1], rstd

            def transpose_128(dst_ap, src_ap):
                """dst[128, n] = src[n, 128].T via PE; dst is SBUF, src SBUF."""
                n = src_ap.shape[0]
                tr = ps.tile([128, 128], F32, tag="ps")
                nc.tensor.transpose(tr[:, 0:n], src_ap, ident[0:n, 0:n])
                nc.vector.tensor_copy(dst_ap, tr[:, 0:n])

            for rep in range(reps):
                # =========== embed + ln0 -> h ===========
                h_t = big.tile([128, RT, D], F32, tag="h")
                xT = big.tile([128, KT, RT * 128], F32R, tag="xT")
                for rt in range(RT):
                    xrow = xnp.tile([128, D], F32, tag="xrow")
                    nc.sync.dma_start(
                        out=xrow,
                        in_=x_in.rearrange("b a d -> a b d")[4 * rt:4 * rt + 4])
                    for dt in range(KT):
                        transpose_128(xT[:, dt, rt * 128:(rt + 1) * 128],
                                      xrow[:, dt * 128:(dt + 1) * 128])
                for ag in range(4):
                    for j in range(4):
                        a = ag * 4 + j
                        wt = wpool.tile([128, KT, D], F32R, tag="w")
                        nc.sync.dma_start(
                            out=wt,
                            in_=w_emb[a].rearrange("(kt p) n -> p kt n",
                                                   p=128).bitcast(F32R))
                        pe = ps.tile([32, D], F32, tag="ps")
                        for kt in range(KT):
                            nc.tensor.matmul(
                                pe, xT[:, kt, a * 32:(a + 1) * 32],
                                wt[:, kt, :],
                                start=(kt == 0), stop=(kt == KT - 1))
                        mean, rstd = ln_stats(pe)
                        nc.vector.tensor_scalar(
                            h_t[32 * j:32 * j + 32, ag, :], pe, mean, rstd,
                            op0=ALU.subtract, op1=ALU.mult)
                    hsl = h_t[:, ag, :]
                    nc.vector.tensor_mul(hsl, hsl, g0bc)
                    nc.vector.tensor_add(hsl, hsl, b0bc)

                if rep == reps - 1:
                    tap_dma("t_h0", h_t,
                            dst_slice=lambda t: t.rearrange(
                                "(t p) d -> p t d", p=128))

                # =========== steps ===========
                for s in range(STEPS):
                    # ---- xn = LN(h; ln1), transposed to xnT (d-major) ----
                    xnT = big.tile([128, KT, RT * 128], F32R, tag="xnT")
                    for rt in range(RT):
                        xn_rt = xnp.tile([128, D], F32, tag="xn_rt")
                        mean, rstd = ln_stats(h_t[:, rt, :])
                        nc.vector.tensor_scalar(xn_rt, h_t[:, rt, :], mean,
                                                rstd, op0=ALU.subtract,
                                                op1=ALU.mult)
                        nc.vector.tensor_mul(xn_rt, xn_rt, g1bc)
                        nc.vector.tensor_add(xn_rt, xn_rt, b1bc)
                        if s == 0 and rep == reps - 1:
                            tap_dma("t_xn0", xn_rt,
                                    dst_slice=lambda t, rt=rt: t.rearrange(
                                        "(t p) d -> p t d", p=128)[:, rt, :])
                        for dt in range(KT):
                            transpose_128(xnT[:, dt, rt * 128:(rt + 1) * 128],
                                          xn_rt[:, dt * 128:(dt + 1) * 128])

                    # ---- q/k/v projections (agent-sharded) ----
                    q_sb = big.tile([128, RT, D], F32, tag="q_sb")
                    k_sb = big.tile([128, RT, D], F32, tag="k_sb")
                    v_sb = big.tile([128, RT, D], F32, tag="v_sb")
                    for ag in range(4):
                        for which, wsrc, dst in (("q", w_q, q_sb),
                                                 ("k", w_k, k_sb)):
                            for j in range(4):
                                a = ag * 4 + j
                                wt = wpool.tile([128, KT, D], F32R, tag="w")
                                nc.sync.dma_start(
                                    out=wt,
                                    in_=wsrc[a].rearrange(
                                        "(kt p) n -> p kt n",
                                        p=128).bitcast(F32R))
                                pp = ps.tile([32, D], F32, tag="ps")
                                for kt in range(KT):
                                    nc.tensor.matmul(
                                        pp, xnT[:, kt, a * 32:(a + 1) * 32],
                                        wt[:, kt, :],
                                        start=(kt == 0), stop=(kt == KT - 1))
                                osl = dst[32 * j:32 * j + 32, ag, :]
                                if which == "q":
                                    # fold in the 1/sqrt(DH) attention scale
                                    nc.vector.tensor_scalar_mul(
                                        osl, pp, DH ** -0.5)
                                else:
                                    nc.vector.tensor_copy(osl, pp)
                    for rt in range(RT):
                        pv = ps.tile([128, D], F32, tag="ps")
                        for kt in range(KT):
                            nc.tensor.matmul(
                                pv, xnT[:, kt, rt * 128:(rt + 1) * 128],
                                wv_t[:, kt, :],
                                start=(kt == 0), stop=(kt == KT - 1))
                        nc.vector.tensor_copy(v_sb[:, rt, :], pv)
                    if s == 0 and rep == reps - 1:
                        for nm, tt in (("t_q0", q_sb), ("t_k0", k_sb),
                                       ("t_v0", v_sb)):
                            tap_dma(nm, tt,
                                    dst_slice=lambda t: t.rearrange(
                                        "(t p) d -> p t d", p=128))

                    # ---- transpose q/k to d-major for the exchange ----
                    qT = big.tile([128, KT, RT * 128], F32, tag="qT")
                    kT = big.tile([128, KT, RT * 128], F32, tag="kT")
                    for src, dst in ((q_sb, qT), (k_sb, kT)):
                        for rt in range(RT):
                            for dt in range(KT):
                                dv = dst[:, dt, :].rearrange(
                                    "p (b a) -> p a b",
                                    a=AL)[:, 4 * rt:4 * rt + 4, :]
                                tr = ps.tile([128, 128], F32, tag="ps")
                                nc.tensor.transpose(
                                    tr, src[:, rt, dt * 128:(dt + 1) * 128],
                                    ident)
                                nc.vector.tensor_copy(
                                    dv, tr.rearrange("p (a b) -> p a b", b=32))

                    # ---- AllToAll: q/k (d-major) + v to batch sharding ----
                    # layout [core_chunk, b_local, {q,k,v}, 8192]
                    # q/k chunk payload: [512 d, 16 a]; v payload: [16 a, 512 d]
                    qkv_send = dpool.tile([NC, BL, 3, D * AL], F32,
                                          tag="qkv_send")
                    qkv_recv = dpool.tile([NC, BL, 3, D * AL], F32,
                                          tag="qkv_recv")
                    for ti, src in ((0, qT), (1, kT)):
                        reg = qkv_send[:, :, ti, :].rearrange(
                            "c b (d a) -> d c b a", a=AL)
                        for dt in range(KT):
                            for c in range(NC):
                                nc.sync.dma_start(
                                    out=reg[dt * 128:(dt + 1) * 128, c].opt(),
                                    in_=src[:, dt, c * 64:(c + 1) * 64].opt())
                    vreg = qkv_send[:, :, 2, :].rearrange(
                        "c b (a d) -> a (c b) d", d=D)
                    for rt in range(RT):
                        for j in range(4):
                            a = rt * 4 + j
                            nc.sync.dma_start(
                                out=vreg[a],
                                in_=v_sb[32 * j:32 * (j + 1), rt, :])
                    nc.gpsimd.collective_compute(
                        "AllToAll", ALU.bypass,
                        replica_groups=[list(range(NC))],
                        ins=[qkv_send.opt()], outs=[qkv_recv.opt()])

                    # ---- attention (batch-sharded: BL batches, all heads) ----
                    xn_send = dpool.tile([NC, AL, BL, D], F32, tag="xn_send")
                    xn_recv = dpool.tile([NC, AL, BL, D], F32, tag="xn_recv")
                    for bl in range(BL):
                        qT_b = att.tile([128, KT, A], F32, tag="qT_b")
                        kT_b = att.tile([128, KT, A], F32, tag="kT_b")
                        v_b = att.tile([128, H, DH + 1], F32, tag="v_b")
                        for ti, dst in ((0, qT_b), (1, kT_b)):
                            for dt in range(KT):
                                nc.sync.dma_start(
                                    out=dst[:, dt, :].opt(),
                                    in_=qkv_recv[:, bl, ti, :].rearrange(
                                        "s (t p a) -> t p s a",
                                        t=KT, p=128)[dt].opt())
                        for c in range(NC):
                            nc.sync.dma_start(
                                out=v_b[c * AL:(c + 1) * AL, :, 0:DH],
                                in_=qkv_recv[c, bl, 2, :].rearrange(
                                    "(a h w) -> a h w", h=H, w=DH))
                        nc.vector.memset(v_b[:, :, DH:DH + 1], 1.0)
                        if s == 0 and bl == 0 and rep == reps - 1:
                            tap_dma("t_qTb0", qT_b.rearrange("p t a -> p (t a)"))
                            tap_dma("t_kTb0", kT_b.rearrange("p t a -> p (t a)"))
                            tap_dma("t_vb0", v_b.rearrange("p h w -> p (h w)"))

                        out_b = att.tile([128, D], F32, tag="out_b")
                        for hh in range(H):
                            dt, half = divmod(hh, 2)
                            half *= 64
                            ps_s = ps.tile([A, A], F32, tag="ps")
                            nc.tensor.matmul(ps_s,
                                             kT_b[half:half + 64, dt, :],
                                             qT_b[half:half + 64, dt, :],
                                             start=True, stop=True)
                            e_sb = att.tile([A, A], F32, tag="e_sb")
                            nc.vector.tensor_add(e_sb, ps_s, ct_sb)
                            nc.scalar.activation(e_sb, e_sb, AF.Exp)
                            ps_o = ps.tile([DH + 1, A], F32, tag="ps")
                            nc.tensor.matmul(ps_o, v_b[:, hh, :], e_sb,
                                             start=True, stop=True)
                            if s == 0 and bl == 0 and hh == 0 and rep == reps - 1:
                                tap_dma("t_eT0", e_sb)
                            o_sb = att.tile([DH + 1, A], F32, tag="o_sb")
                            nc.vector.tensor_copy(o_sb, ps_o)
                            ps_t = ps.tile([128, DH + 1], F32, tag="ps")
                            nc.tensor.transpose(ps_t, o_sb,
                                                ident[0:DH + 1, 0:DH + 1])
                            rc = sm.tile([128, 1], F32, tag="rc")
                            nc.vector.reciprocal(rc, ps_t[:, DH:DH + 1])
                            nc.vector.tensor_scalar_mul(
                                out_b[:, hh * DH:(hh + 1) * DH],
                                ps_t[:, 0:DH], rc)
                        # LN1 on attention output, still batch-sharded
                        xn_b = att.tile([128, D], F32, tag="xn_b")
                        mean, rstd = ln_stats(out_b)
                        nc.vector.tensor_scalar(xn_b, out_b, mean, rstd,
                                                op0=ALU.subtract, op1=ALU.mult)
                        nc.vector.tensor_mul(xn_b, xn_b, g1bc)
                        nc.vector.tensor_add(xn_b, xn_b, b1bc)
                        if s == 0 and bl == 0 and rep == reps - 1:
                            tap_dma("t_ob0", out_b)
                            tap_dma("t_xnb0", xn_b)
                        nc.sync.dma_start(
                            out=xn_send.rearrange(
                                "c a b d -> (c a) b d")[:, bl, :],
                            in_=xn_b)

                    nc.gpsimd.collective_compute(
                        "AllToAll", ALU.bypass,
                        replica_groups=[list(range(NC))],
                        ins=[xn_send.opt()], outs=[xn_recv.opt()])

                    # ---- fwd MLP (agent-sharded) + residual ----
                    xnTf = big.tile([128, KT, RT * 128], F32R, tag="xnTf")
                    for rt in range(RT):
                        xf = xnp.tile([128, D], F32, tag="xf")
                        for al in range(4):
                            nc.sync.dma_start(
                                out=xf[al * 32:(al + 1) * 32, :],
                                in_=xn_recv[:, 4 * rt + al].opt())
                        if s == 0 and rep == reps - 1:
                            tap_dma("t_xf0", xf,
                                    dst_slice=lambda t, rt=rt: t.rearrange(
                                        "(t p) d -> p t d", p=128)[:, rt, :])
                        for dt in range(KT):
                            transpose_128(xnTf[:, dt, rt * 128:(rt + 1) * 128],
                                          xf[:, dt * 128:(dt + 1) * 128])
                    for ag in range(4):
                        st_sb = xnp.tile([128, D], F32, tag="st_sb")
                        for j in range(4):
                            a = ag * 4 + j
                            wt = wfpool.tile([128, KT, D], F32R, tag="wf")
                            nc.sync.dma_start(
                                out=wt,
                                in_=w_f[a].rearrange("(kt p) n -> p kt n",
                                                     p=128).bitcast(F32R))
                            pf = ps.tile([32, D], F32, tag="ps")
                            for kt in range(KT):
                                nc.tensor.matmul(
                                    pf, xnTf[:, kt, a * 32:(a + 1) * 32],
                                    wt[:, kt, :],
                                    start=(kt == 0), stop=False)
                            nc.tensor.matmul(
                                pf, ones_r, bfs[:, a, :],
                                start=False, stop=True)
                            nc.scalar.activation(st_sb[32 * j:32 * j + 32, :],
                                                 pf, AF.Silu)
                        if s == 0 and rep == reps - 1:
                            tap_dma("t_st0", st_sb,
                                    dst_slice=lambda t, ag=ag: t.rearrange(
                                        "(t p) d -> p t d", p=128)[:, ag, :])
                        nc.vector.tensor_add(h_t[:, ag, :], h_t[:, ag, :],
                                             st_sb)

                if rep == reps - 1:
                    nc.sync.dma_start(
                        out=h_out.rearrange("(t p) d -> p t d", p=128),
                        in_=h_t)

    nc.compile()
    return nc


_CACHE = {}


def _get(reps=1):
    if reps not in _CACHE:
        _CACHE[reps] = build(reps)
    return _CACHE[reps]


def _in_maps(x, W_embed, ln0_g, ln0_b, W_q, W_k, W_v, W_fwd, b_fwd,
             ln1_g, ln1_b, connect):
    f = lambda t: np.ascontiguousarray(np.asarray(t, dtype=np.float32))
    connect_t = f(np.asarray(connect).T)
    maps = []
    for c in range(NC):
        sl = slice(c * AL, (c + 1) * AL)
        maps.append({
            "x": f(np.asarray(x)[:, sl, :]),
            "w_embed": f(np.asarray(W_embed)[sl]),
            "w_q": f(np.asarray(W_q)[sl]),
            "w_k": f(np.asarray(W_k)[sl]),
            "w_fwd": f(np.asarray(W_fwd)[sl]),
            "w_v": f(W_v),
            "b_fwd": f(np.asarray(b_fwd)[sl]),
            "ln0_g": f(ln0_g), "ln0_b": f(ln0_b),
            "ln1_g": f(ln1_g), "ln1_b": f(ln1_b),
            "connect_t": connect_t,
        })
    return maps


def _assemble(results):
    out = np.empty((B, A, D), np.float32)
    for c in range(NC):
        hc = results[c]["h_out"].reshape(AL, B, D)
        out[:, c * AL:(c + 1) * AL, :] = hc.transpose(1, 0, 2)
    return out


def kernel(**inputs) -> np.ndarray:
    nc = _get(1)
    maps = _in_maps(**inputs)
    res = bass_utils.run_bass_kernel_spmd(nc, maps, core_ids=list(range(NC)))
    return _assemble(res.results)


def run_reps(reps, **inputs):
    """Timing helper: same computation repeated `reps` times on-device."""
    nc = _get(reps)
    maps = _in_maps(**inputs)
    res = bass_utils.run_bass_kernel_spmd(nc, maps, core_ids=list(range(NC)))
    return _assemble(res.results)


# revision 29
# speedup vs baseline: 477.6096x; 477.6096x over previous
"""Distributed Dot-GAT kernel for 8 Trainium2 NeuronCores.

Sharding:
  - Per-agent weights (W_embed/W_q/W_k/W_fwd, b_fwd) sharded 16 agents/core.
  - Embed, LN, q/k/v projections, fwd MLP run agent-sharded (b-major rows
    (a_loc, b), free = d).
  - Attention is sharded by batch (4 of 32 per core): q/k/v are exchanged
    b-major with a single AllToAll per step (q/k transposed to d-major on the
    attention side); attention output (after LN) returns to agent sharding
    with a second AllToAll.
  - Projection matmuls run as float32r (1 cyc/row at N=512); attention
    matmuls stay fp32 (no speed difference at N=128).
  - DMA issue is spread across the SP and Activation HWDGE queues plus
    GPSIMD SWDGE to avoid serializing on one sequencer.

All cores run the same program (SPMD); no core-id-dependent addressing.
"""
import numpy as np

import concourse.bass as bass
import concourse.bacc as bacc
import concourse.tile as tile
import concourse.mybir as mybir
from concourse import bass_utils
from concourse.masks import make_identity

B, A, D = 32, 128, 512
H, DH = 8, 64
NC = 8
AL = A // NC          # agents per core
BL = B // NC          # batches per core (attention phase)
STEPS = 3
KT = D // 128         # contraction tiles
RT = (AL * B) // 128  # row tiles of the (a_loc, b) x d state
EPS = 1e-5
F32 = mybir.dt.float32
F32R = mybir.dt.float32r
AF = mybir.ActivationFunctionType
ALU = mybir.AluOpType

DEBUG = False
NO_COLLECTIVE = False
LN_AFFINE = False     # apply ln gamma/beta (setup_inputs uses identity)
CONNECT_VAR = False   # connect varies along the softmax axis (it is all-ones;
                      # a row-constant additive mask cancels in softmax)


def build(reps=1, steps=STEPS):
    nc = bacc.Bacc("TRN2", target_bir_lowering=False, debug=False,
                   num_devices=NC)
    taps = {}

    def tap(name, shape):
        if DEBUG and name not in taps:
            taps[name] = nc.dram_tensor(name, shape, F32,
                                        kind="ExternalOutput").ap()
        return taps.get(name)

    def tap_dma(name, src_ap, dst_slice=None):
        if not DEBUG:
            return
        t = taps[name]
        nc.sync.dma_start(out=t if dst_slice is None else dst_slice(t),
                          in_=src_ap)

    x_in = nc.dram_tensor("x", [B, AL, D], F32, kind="ExternalInput").ap()
    w_emb = nc.dram_tensor("w_embed", [AL, D, D], F32, kind="ExternalInput").ap()
    w_q = nc.dram_tensor("w_q", [AL, D, D], F32, kind="ExternalInput").ap()
    w_k = nc.dram_tensor("w_k", [AL, D, D], F32, kind="ExternalInput").ap()
    w_f = nc.dram_tensor("w_fwd", [AL, D, D], F32, kind="ExternalInput").ap()
    w_v = nc.dram_tensor("w_v", [D, D], F32, kind="ExternalInput").ap()
    bf_in = nc.dram_tensor("b_fwd", [AL, D], F32, kind="ExternalInput").ap()
    g0_in = nc.dram_tensor("ln0_g", [D], F32, kind="ExternalInput").ap()
    b0_in = nc.dram_tensor("ln0_b", [D], F32, kind="ExternalInput").ap()
    g1_in = nc.dram_tensor("ln1_g", [D], F32, kind="ExternalInput").ap()
    b1_in = nc.dram_tensor("ln1_b", [D], F32, kind="ExternalInput").ap()
    ct_in = nc.dram_tensor("connect_t", [A, A], F32, kind="ExternalInput").ap()
    h_out = nc.dram_tensor("h_out", [AL * B, D], F32, kind="ExternalOutput").ap()
    if True:
        tap("t_h0", [AL * B, D])
        tap("t_xn0", [AL * B, D])
        tap("t_q0", [AL * B, D])
        tap("t_k0", [AL * B, D])
        tap("t_v0", [AL * B, D])
        tap("t_qTb0", [128, KT * A])
        tap("t_kTb0", [128, KT * A])
        tap("t_vb0", [128, H * DH])
        tap("t_eT0", [A, A])
        tap("t_ob0", [128, D])
        tap("t_xnb0", [128, D])
        tap("t_xf0", [AL * B, D])
        tap("t_st0", [AL * B, D])

    with tile.TileContext(nc) as tc:
        with (
            tc.tile_pool(name="const", bufs=1) as const,
            tc.tile_pool(name="w", bufs=3) as wpool,
            tc.tile_pool(name="wf", bufs=2) as wfpool,
            tc.tile_pool(name="big", bufs=1) as big,
            tc.tile_pool(name="xnp", bufs=2) as xnp,
            tc.tile_pool(name="att", bufs=2) as att,
            tc.tile_pool(name="attb", bufs=4) as attb,
            tc.tile_pool(name="sm", bufs=4) as sm,
            tc.tile_pool(name="ps", bufs=8, space="PSUM") as ps,
            tc.tile_pool(name="dram", bufs=2, space="DRAM") as dpool,
        ):
            # ---- constants ----
            ident = const.tile([128, 128], F32)
            make_identity(nc, ident)
            if CONNECT_VAR:
                ct4 = const.tile([A, BL * A], F32)
                for bl in range(BL):
                    nc.sync.dma_start(out=ct4[:, bl * A:(bl + 1) * A],
                                      in_=ct_in)
            if LN_AFFINE:
                bcast = []
                for t_in in (g0_in, b0_in, g1_in, b1_in):
                    t_sb = const.tile([128, D], F32,
                                      tag=f"bc_{t_in.tensor.name}")
                    nc.sync.dma_start(out=t_sb,
                                      in_=t_in[None].to_broadcast([128, D]))
                    bcast.append(t_sb)
                g0bc, b0bc, g1bc, b1bc = bcast
            wv_t = const.tile([128, KT, D], F32R)
            nc.sync.dma_start(
                out=wv_t,
                in_=w_v.rearrange("(kt p) n -> p kt n", p=128).bitcast(F32R))
            eps_t = const.tile([128, 1], F32)
            nc.vector.memset(eps_t, EPS)
            ones_t = const.tile([128, 1], F32)
            nc.vector.memset(ones_t, 1.0)
            bias_t = const.tile([128, RT, D], F32)
            for rt in range(RT):
                nc.sync.dma_start(
                    out=bias_t[:, rt, :],
                    in_=bass.AP(tensor=bf_in.tensor, offset=4 * rt * D,
                                ap=[[D, 4], [0, 32], [1, D]]))

            def ln_inplace(dst_ap, src_ap, affine):
                """dst = LN(src) over the free dim; dst/src [p, 512]."""
                p = src_ap.shape[0]
                st = sm.tile([128, 6], F32, tag="st")
                mv = sm.tile([128, 2], F32, tag="mv")
                sq = sm.tile([128, 1], F32, tag="sq")
                rstd = sm.tile([128, 1], F32, tag="rstd")
                st, mv, sq, rstd = st[0:p], mv[0:p], sq[0:p], rstd[0:p]
                nc.vector.bn_stats(st, src_ap)
                nc.vector.bn_aggr(mv, st)
                nc.scalar.activation(sq, mv[:, 1:2], AF.Sqrt, bias=eps_t[0:p])
                nc.vector.reciprocal(rstd, sq)
                nc.vector.tensor_scalar(dst_ap, src_ap, mv[:, 0:1], rstd,
                                        op0=ALU.subtract, op1=ALU.mult)
                if LN_AFFINE:
                    g, b = affine
                    nc.vector.tensor_mul(dst_ap, dst_ap, g[0:p])
                    nc.vector.tensor_add(dst_ap, dst_ap, b[0:p])

            def transpose_128(dst_ap, src_ap):
                """dst[128, n] = src[n, 128].T via PE; psum bounce + DVE copy."""
                n = src_ap.shape[0]
                tr = ps.tile([128, 128], F32, tag="ps")
                nc.tensor.transpose(tr[:, 0:n], src_ap, ident[0:n, 0:n])
                nc.vector.tensor_copy(dst_ap, tr[:, 0:n])

            def load_w2(pool, tag, wsrc, pair, eng=None):
                """Load weights for agents (2*pair, 2*pair+1): [128, 2*KT, D]."""
                wt = pool.tile([128, 2 * KT, D], F32R, tag=tag)
                (eng or nc.gpsimd).dma_start(
                    out=wt,
                    in_=wsrc[2 * pair:2 * pair + 2].rearrange(
                        "a (kt p) n -> p (a kt) n", p=128).bitcast(F32R))
                return wt

            aff0 = (g0bc, b0bc) if LN_AFFINE else None
            aff1 = (g1bc, b1bc) if LN_AFFINE else None

            for rep in range(reps):
                # =========== embed + ln0 -> h ===========
                h_t = big.tile([128, RT, D], F32, tag="h")
                xT = big.tile([128, KT, RT * 128], F32R, tag="xTsh")
                for rt in range(RT):
                    xrow = xnp.tile([128, D], F32, tag="xrow")
                    nc.sync.dma_start(
                        out=xrow,
                        in_=x_in.rearrange("b a d -> a b d")[4 * rt:4 * rt + 4])
                    for dt in range(KT):
                        transpose_128(xT[:, dt, rt * 128:(rt + 1) * 128],
                                      xrow[:, dt * 128:(dt + 1) * 128])
                for pair in range(AL // 2):
                    wt = load_w2(wpool, "w", w_emb, pair)
                    for j2 in range(2):
                        a = 2 * pair + j2
                        pe = ps.tile([32, D], F32, tag="ps")
                        for kt in range(KT):
                            nc.tensor.matmul(
                                pe, xT[:, kt, a * 32:(a + 1) * 32],
                                wt[:, j2 * KT + kt, :],
                                start=(kt == 0), stop=(kt == KT - 1))
                        j = a % 4
                        nc.vector.tensor_copy(
                            h_t[32 * j:32 * j + 32, a // 4, :], pe)
                for ag in range(4):
                    hsl = h_t[:, ag, :]
                    ln_inplace(hsl, hsl, aff0)

                # =========== steps ===========
                for s in range(steps):
                    if rep == reps - 1 and s == 0:
                        tap_dma("t_h0", h_t,
                                dst_slice=lambda t: t.rearrange(
                                    "(t p) d -> p t d", p=128))
                    # ---- xn = LN(h; ln1), transposed to xnT (d-major) ----
                    xnT = big.tile([128, KT, RT * 128], F32R, tag="xTsh")
                    for rt in range(RT):
                        xn_rt = xnp.tile([128, D], F32, tag="xn_rt")
                        ln_inplace(xn_rt, h_t[:, rt, :], aff1)
                        if s == 0 and rep == reps - 1:
                            tap_dma("t_xn0", xn_rt,
                                    dst_slice=lambda t, rt=rt: t.rearrange(
                                        "(t p) d -> p t d", p=128)[:, rt, :])
                        for dt in range(KT):
                            transpose_128(xnT[:, dt, rt * 128:(rt + 1) * 128],
                                          xn_rt[:, dt * 128:(dt + 1) * 128])

                    # ---- q/k/v projections (agent-sharded) ----
                    q_sb = big.tile([128, RT, D], F32, tag="q_sb")
                    k_sb = big.tile([128, RT, D], F32, tag="k_sb")
                    v_sb = big.tile([128, RT, D], F32, tag="v_sb")
                    for which, wsrc, dst in (("q", w_q, q_sb), ("k", w_k, k_sb)):
                        for pair in range(AL // 2):
                            wt = load_w2(wpool, "w", wsrc, pair)
                            for j2 in range(2):
                                a = 2 * pair + j2
                                pp = ps.tile([32, D], F32, tag="ps")
                                for kt in range(KT):
                                    nc.tensor.matmul(
                                        pp, xnT[:, kt, a * 32:(a + 1) * 32],
                                        wt[:, j2 * KT + kt, :],
                                        start=(kt == 0), stop=(kt == KT - 1))
                                osl = dst[32 * (a % 4):32 * (a % 4) + 32,
                                          a // 4, :]
                                if which == "q":
                                    # fold in the 1/sqrt(DH) attention scale
                                    nc.vector.tensor_scalar_mul(
                                        osl, pp, DH ** -0.5)
                                else:
                                    nc.vector.tensor_copy(osl, pp)
                    for rt in range(RT):
                        pv = ps.tile([128, D], F32, tag="ps")
                        for kt in range(KT):
                            nc.tensor.matmul(
                                pv, xnT[:, kt, rt * 128:(rt + 1) * 128],
                                wv_t[:, kt, :],
                                start=(kt == 0), stop=(kt == KT - 1))
                        nc.vector.tensor_copy(v_sb[:, rt, :], pv)
                    if s == 0 and rep == reps - 1:
                        for nm, tt in (("t_q0", q_sb), ("t_k0", k_sb),
                                       ("t_v0", v_sb)):
                            tap_dma(nm, tt,
                                    dst_slice=lambda t: t.rearrange(
                                        "(t p) d -> p t d", p=128))

                    # ---- AllToAll: q/k/v (b-major rows) to batch sharding ----
                    # chunk payload per (dst_core, b_local): 3 x [16 a, 512 d]
                    qkv_send = dpool.tile([NC, BL, 3, D * AL], F32,
                                          tag="qkv_send")
                    qkv_recv = dpool.tile([NC, BL, 3, D * AL], F32,
                                          tag="qkv_recv")
                    for ti, src in ((0, q_sb), (1, k_sb), (2, v_sb)):
                        reg = qkv_send[:, :, ti, :].rearrange(
                            "c b (a d) -> a (c b) d", d=D)
                        for rt in range(RT):
                            nc.sync.dma_start(out=reg[4 * rt:4 * rt + 4],
                                              in_=src[:, rt, :])
                    if NO_COLLECTIVE:
                        nc.sync.dma_start(out=qkv_recv.opt(),
                                          in_=qkv_send.opt())
                    else:
                        nc.gpsimd.collective_compute(
                            "AllToAll", ALU.bypass,
                            replica_groups=[list(range(NC))],
                            ins=[qkv_send.opt()], outs=[qkv_recv.opt()])

                    # ---- attention (batch-sharded: BL batches, all heads) ----
                    xn_send = dpool.tile([NC, AL, BL, D], F32, tag="xn_send")
                    xn_recv = dpool.tile([NC, AL, BL, D], F32, tag="xn_recv")
                    qTbs, kTbs, vbs, outbs = [], [], [], []
                    for bl in range(BL):
                        for ti, lst, name in ((0, qTbs, "qT_b"),
                                              (1, kTbs, "kT_b")):
                            srcb = att.tile([128, D], F32, tag=f"src_{name}")
                            nc.sync.dma_start(
                                out=srcb,
                                in_=qkv_recv[:, bl, ti, :].rearrange(
                                    "s (a d) -> s a d", d=D).opt())
                            dstT = attb.tile([128, KT, A], F32, tag=name)
                            for dt in range(KT):
                                transpose_128(
                                    dstT[:, dt, :],
                                    srcb[:, dt * 128:(dt + 1) * 128])
                            lst.append(dstT)
                        v_b = attb.tile([128, H, DH], F32, tag="v_b")
                        nc.sync.dma_start(
                            out=v_b,
                            in_=qkv_recv[:, bl, 2, :].rearrange(
                                "s (a hw) -> s a hw", hw=H * DH).opt())
                        vbs.append(v_b)
                        out_b = attb.tile([128, D], F32, tag="out_b")
                        outbs.append(out_b)
                    if s == 0 and rep == reps - 1:
                        tap_dma("t_qTb0", qTbs[0].rearrange("p t a -> p (t a)"))
                        tap_dma("t_kTb0", kTbs[0].rearrange("p t a -> p (t a)"))
                        tap_dma("t_vb0", vbs[0].rearrange("p h w -> p (h w)"))

                    for hh in range(H):
                        dth, half = divmod(hh, 2)
                        half *= 64
                        ps_s = ps.tile([A, BL * A], F32, tag="ps")
                        for bl in range(BL):
                            nc.tensor.matmul(ps_s[:, bl * A:(bl + 1) * A],
                                             kTbs[bl][half:half + 64, dth, :],
                                             qTbs[bl][half:half + 64, dth, :],
                                             start=True, stop=True)
                        e_sb = att.tile([A, BL * A], F32, tag="e_sb")
                        if CONNECT_VAR:
                            nc.vector.tensor_add(e_sb, ps_s, ct4)
                            nc.scalar.activation(e_sb, e_sb, AF.Exp)
                        else:
                            nc.scalar.activation(e_sb, ps_s, AF.Exp)
                        if s == 0 and hh == 0 and rep == reps - 1:
                            tap_dma("t_eT0", e_sb[:, 0:A])
                        ps_o = ps.tile([DH + 1, BL * A], F32, tag="ps")
                        for bl in range(BL):
                            nc.tensor.matmul(ps_o[0:DH, bl * A:(bl + 1) * A],
                                             vbs[bl][:, hh, :],
                                             e_sb[:, bl * A:(bl + 1) * A],
                                             start=True, stop=True)
                        nc.tensor.matmul(ps_o[DH:DH + 1, :], ones_t, e_sb,
                                         start=True, stop=True)
                        o_sb = att.tile([DH + 1, BL * A], F32, tag="o_sb")
                        nc.vector.tensor_copy(o_sb, ps_o)
                        ps_t = ps.tile([128, BL, DH + 1], F32, tag="ps")
                        for bl in range(BL):
                            nc.tensor.transpose(ps_t[:, bl, :],
                                                o_sb[:, bl * A:(bl + 1) * A],
                                                ident[0:DH + 1, 0:DH + 1])
                        rc = sm.tile([128, BL], F32, tag="rc")
                        nc.vector.reciprocal(rc, ps_t[:, :, DH:DH + 1])
                        for bl in range(BL):
                            nc.vector.tensor_scalar_mul(
                                outbs[bl][:, hh * DH:(hh + 1) * DH],
                                ps_t[:, bl, 0:DH], rc[:, bl:bl + 1])

                    for bl in range(BL):
                        out_b = outbs[bl]
                        # LN1 on attention output, still batch-sharded
                        xn_b = att.tile([128, D], F32, tag="xn_b")
                        ln_inplace(xn_b, out_b, aff1)
                        if s == 0 and bl == 0 and rep == reps - 1:
                            tap_dma("t_ob0", out_b)
                            tap_dma("t_xnb0", xn_b)
                        nc.sync.dma_start(
                            out=xn_send.rearrange(
                                "c a b d -> (c a) b d")[:, bl, :],
                            in_=xn_b)

                    if NO_COLLECTIVE:
                        nc.sync.dma_start(out=xn_recv.opt(),
                                          in_=xn_send.opt())
                    else:
                        nc.gpsimd.collective_compute(
                            "AllToAll", ALU.bypass,
                            replica_groups=[list(range(NC))],
                            ins=[xn_send.opt()], outs=[xn_recv.opt()])

                    # ---- fwd MLP (agent-sharded) + residual ----
                    xnTf = big.tile([128, KT, RT * 128], F32R, tag="xTsh")
                    for rt in range(RT):
                        xf = xnp.tile([128, D], F32, tag="xf")
                        for al in range(4):
                            nc.sync.dma_start(
                                out=xf[al * 32:(al + 1) * 32, :],
                                in_=xn_recv[:, 4 * rt + al].opt())
                        if s == 0 and rep == reps - 1:
                            tap_dma("t_xf0", xf,
                                    dst_slice=lambda t, rt=rt: t.rearrange(
                                        "(t p) d -> p t d", p=128)[:, rt, :])
                        for dt in range(KT):
                            transpose_128(xnTf[:, dt, rt * 128:(rt + 1) * 128],
                                          xf[:, dt * 128:(dt + 1) * 128])
                    for ag in range(4):
                        st_sb = xnp.tile([128, D], F32, tag="st_sb")
                        for pair in (2 * ag, 2 * ag + 1):
                            wt = load_w2(wfpool, "wf", w_f, pair, eng=nc.scalar)
                            for j2 in range(2):
                                a = 2 * pair + j2
                                pf = ps.tile([32, D], F32, tag="ps")
                                for kt in range(KT):
                                    nc.tensor.matmul(
                                        pf, xnTf[:, kt, a * 32:(a + 1) * 32],
                                        wt[:, j2 * KT + kt, :],
                                        start=(kt == 0), stop=(kt == KT - 1))
                                j = a % 4
                                nc.vector.tensor_add(
                                    st_sb[32 * j:32 * j + 32, :], pf,
                                    bias_t[32 * j:32 * j + 32, ag, :])
                        nc.scalar.activation(st_sb, st_sb, AF.Silu)
                        if s == 0 and rep == reps - 1:
                            tap_dma("t_st0", st_sb,
                                    dst_slice=lambda t, ag=ag: t.rearrange(
                                        "(t p) d -> p t d", p=128)[:, ag, :])
                        nc.vector.tensor_add(h_t[:, ag, :], h_t[:, ag, :],
                                             st_sb)

                if rep == reps - 1:
                    nc.sync.dma_start(
                        out=h_out.rearrange("(t p) d -> p t d", p=128),
                        in_=h_t)

    nc.compile()
    return nc


_CACHE = {}


def _get(reps=1):
    key = (reps, DEBUG, NO_COLLECTIVE, LN_AFFINE, CONNECT_VAR)
    if key not in _CACHE:
        _CACHE[key] = build(reps)
    return _CACHE[key]


def _in_maps(x, W_embed, ln0_g, ln0_b, W_q, W_k, W_v, W_fwd, b_fwd,
             ln1_g, ln1_b, connect):
    f = lambda t: np.ascontiguousarray(np.asarray(t, dtype=np.float32))
    connect_t = f(np.asarray(connect).T)
    maps = []
    for c in range(NC):
        sl = slice(c * AL, (c + 1) * AL)
        maps.append({
            "x": f(np.asarray(x)[:, sl, :]),
            "w_embed": f(np.asarray(W_embed)[sl]),
            "w_q": f(np.asarray(W_q)[sl]),
            "w_k": f(np.asarray(W_k)[sl]),
            "w_fwd": f(np.asarray(W_fwd)[sl]),
            "w_v": f(W_v),
            "b_fwd": f(np.asarray(b_fwd)[sl]),
            "ln0_g": f(ln0_g), "ln0_b": f(ln0_b),
            "ln1_g": f(ln1_g), "ln1_b": f(ln1_b),
            "connect_t": connect_t,
        })
    return maps


def _assemble(results):
    out = np.empty((B, A, D), np.float32)
    for c in range(NC):
        hc = results[c]["h_out"].reshape(AL, B, D)
        out[:, c * AL:(c + 1) * AL, :] = hc.transpose(1, 0, 2)
    return out


def _needs_affine(inputs):
    return not (np.all(np.asarray(inputs["ln0_g"]) == 1.0)
                and np.all(np.asarray(inputs["ln0_b"]) == 0.0)
                and np.all(np.asarray(inputs["ln1_g"]) == 1.0)
                and np.all(np.asarray(inputs["ln1_b"]) == 0.0))


def _connect_varies(inputs):
    c = np.asarray(inputs["connect"])
    return not np.all(c == c[:, :1])


def kernel(**inputs) -> np.ndarray:
    global LN_AFFINE, CONNECT_VAR
    LN_AFFINE = _needs_affine(inputs)
    CONNECT_VAR = _connect_varies(inputs)
    nc = _get(1)
    maps = _in_maps(**inputs)
    res = bass_utils.run_bass_kernel_spmd(nc, maps, core_ids=list(range(NC)))
    return _assemble(res.results)


def run_reps(reps, **inputs):
    """Timing helper: same computation repeated `reps` times on-device."""
    global LN_AFFINE, CONNECT_VAR
    LN_AFFINE = _needs_affine(inputs)
    CONNECT_VAR = _connect_varies(inputs)
    nc = _get(reps)
    maps = _in_maps(**inputs)
    res = bass_utils.run_bass_kernel_spmd(nc, maps, core_ids=list(range(NC)))
    return _assemble(res.results)
